# revision 1
# baseline (speedup 1.0000x reference)
"""Trainium2 Bass kernel for nn_CrossAttention (B=8, C=512, H=W=32, Lc=1024,
8 heads x 64 dim).

Sharding: data-parallel over batch B across the 8 NeuronCores (1 image/core,
no collectives). Per core, everything is computed in a "feature-on-partitions,
token-on-free-dim" layout so all matmuls contract over SBUF partitions:

  x     [C=512, L=1024]   (natural [C,H,W] flatten)
  ctxT  [C=512, Lc=1024]  (host-transposed context)
  RMS norms: squares (ACT) + ones-matmul partition reduction (PE) ->
    row [1,L]; rsqrt via ln/exp on ACT (exp(-0.5*ln(ssq/C + eps)));
    broadcast back to 128 partitions with a K=1 ones-matmul; fold into
    x/ctx by DVE multiply. RMS scale g and attention scale are folded into
    the weights host-side.
  q = WqT.T @ xn -> [hidden, L];  k = WkT.T @ cn -> [hidden, Lc]
  vT = cn.T @ WvT -> [Lc, hidden] (computed directly, no transposes), stored
    augmented: per head 64 v-columns + a ones column -> PV matmul emits the
    softmax denominator for free as output row 64.
  per head: simT[j,i] = k_h.T @ q_h (K=64); exp on ACT (no max subtraction:
    |sim| <= ~2 with these scales); out_aug[65,i] = sum_j exp[j,:].T-chain.
  softmax denominators are DMA-gathered into one [8, L] tile, reciprocal via
  ln/exp, broadcast by K=1 matmuls, applied by DVE.
  y = WoT.T @ attn_out + bo; output RMS norm same way; + x residual.

All matmuls run as float32r (1 cycle/row; ~1e-4 rel err).
"""

import numpy as np
from contextlib import ExitStack

import concourse.bass as bass
from concourse import bacc
import concourse.mybir as mybir
import concourse.tile as tile
from concourse.bass_utils import run_bass_kernel_spmd

F32 = mybir.dt.float32
F32R = mybir.dt.float32r
AF = mybir.ActivationFunctionType

B, C, H, W = 8, 512, 32, 32
L = H * W  # 1024 query pixels
LC = 1024  # context tokens
HEADS, HD = 8, 64
HID = HEADS * HD  # 512
EPS = 1e-6
NCORES = 8


def build():
    nc = bacc.Bacc("TRN2", target_bir_lowering=False, debug=False,
                   num_devices=NCORES)

    x_d = nc.dram_tensor("x", [C, L], F32R, kind="ExternalInput")
    ct_d = nc.dram_tensor("ctxT", [C, LC], F32R, kind="ExternalInput")
    wq_d = nc.dram_tensor("wq", [C, HID], F32R, kind="ExternalInput")
    wk_d = nc.dram_tensor("wk", [C, HID], F32R, kind="ExternalInput")
    wv_d = nc.dram_tensor("wv", [C, HID], F32R, kind="ExternalInput")
    wo_d = nc.dram_tensor("wo", [HID, C], F32R, kind="ExternalInput")
    ones_d = nc.dram_tensor("ones", [128, 128], F32R, kind="ExternalInput")
    sel_d = nc.dram_tensor("sel", [8, 512], F32R, kind="ExternalInput")
    one_d = nc.dram_tensor("one", [1, 1], F32, kind="ExternalInput")
    bog2_d = nc.dram_tensor("bog2", [C, 2], F32, kind="ExternalInput")
    y_d = nc.dram_tensor("y_out", [C, L], F32, kind="ExternalOutput")

    CT = C // 128  # 4 c-tiles
    NT = L // 512  # 2 n-halves
    JT = LC // 128  # 8 j-tiles
    VW = HD + 1  # 65: per-head v columns + ones column

    with tile.TileContext(nc) as tc, ExitStack() as top:
        pc = top.enter_context(tc.tile_pool(name="const", bufs=1))
        psum = top.enter_context(tc.tile_pool(name="ps", bufs=1, space="PSUM"))

        # ---- constants / whole-kernel tiles ----
        ones_sb = pc.tile([128, 128], F32R)
        nc.sync.dma_start(out=ones_sb, in_=ones_d[:, :])
        sel_sb = pc.tile([8, 512], F32R)
        nc.sync.dma_start(out=sel_sb, in_=sel_d[:, :])
        eps11 = pc.tile([1, 1], F32)
        nc.vector.memset(eps11, EPS)
        x_sb = []
        for t in range(CT):
            xt = pc.tile([128, L], F32R, tag=f"x{t}")
            nc.sync.dma_start(out=xt, in_=x_d[t * 128:(t + 1) * 128, :])
            x_sb.append(xt)
        bo_sb, g2_sb = [], []
        for t in range(CT):
            bt = pc.tile([128, 1], F32, tag=f"bo{t}")
            nc.sync.dma_start(out=bt, in_=bog2_d[t * 128:(t + 1) * 128, 0:1])
            bo_sb.append(bt)
            gt = pc.tile([128, 1], F32, tag=f"g2{t}")
            nc.sync.dma_start(out=gt, in_=bog2_d[t * 128:(t + 1) * 128, 1:2])
            g2_sb.append(gt)

        def rsqrt_row(ps_row, n_feat, sb_row, tag):
            # sb_row <- (ps_row/n_feat + eps)^-0.5 via ln+exp (1-lane ACT)
            lnr = pc.tile([1, L], F32, tag=f"ln_{tag}")
            nc.scalar.activation(out=lnr[:, :], in_=ps_row[:, :], func=AF.Ln,
                                 bias=eps11[:, :], scale=1.0 / n_feat)
            nc.scalar.activation(out=sb_row[:, :], in_=lnr[:, :], func=AF.Exp,
                                 bias=0.0, scale=-0.5)

        pqkv = top.enter_context(tc.tile_pool(name="qkv", bufs=1))
        q_sb = [pqkv.tile([128, L], F32R, tag=f"q{m}", name=f"q{m}") for m in range(CT)]
        k_sb = [pqkv.tile([128, LC], F32R, tag=f"k{m}", name=f"k{m}") for m in range(CT)]
        vT_sb = [pqkv.tile([128, HEADS * VW], F32R, tag=f"vT{j}", name=f"vT{j}")
                 for j in range(JT)]
        pao = top.enter_context(tc.tile_pool(name="aop", bufs=1))
        ao_sb = [pao.tile([128, L], F32R, tag=f"ao{m}", name=f"ao{m}") for m in range(CT)]
        pwo = top.enter_context(tc.tile_pool(name="pwo", bufs=1))
        wo_sb = []
        for t in range(CT):
            wt = pwo.tile([128, C], F32R, tag=f"wo{t}", name=f"wo{t}")
            nc.sync.dma_start(out=wt, in_=wo_d[t * 128:(t + 1) * 128, :])
            wo_sb.append(wt)

        # =========== stage A+B: norms + projections ===========
        with ExitStack() as sab:
            pab = sab.enter_context(tc.tile_pool(name="ab", bufs=1))

            ct_sb, wq_sb, wk_sb, wv_sb = [], [], [], []
            for t in range(CT):
                ctt = pab.tile([128, LC], F32R, tag=f"ct{t}")
                nc.sync.dma_start(out=ctt, in_=ct_d[t * 128:(t + 1) * 128, :])
                ct_sb.append(ctt)
            for name, lst, dram in (("wq", wq_sb, wq_d), ("wk", wk_sb, wk_d),
                                    ("wv", wv_sb, wv_d)):
                for t in range(CT):
                    wt = pab.tile([128, HID], F32R, tag=f"{name}{t}")
                    nc.sync.dma_start(out=wt,
                                      in_=dram[t * 128:(t + 1) * 128, :])
                    lst.append(wt)

            # x-norm then ctx-norm, sequentially (PSUM pressure)
            def rms_normalize(src_tiles, tag, big_tag):
                r_ps = psum.tile([1, L], F32, tag="ou", name=f"r_{tag}")
                for t in range(CT):
                    sq = pab.tile([128, L], F32R, tag="sq", name=f"sq{tag}{t}",
                                  bufs=3)
                    nc.vector.tensor_mul(sq[:, :],
                                         src_tiles[t][:, :].bitcast(F32),
                                         src_tiles[t][:, :].bitcast(F32))
                    for n in range(NT):
                        nc.tensor.matmul(out=r_ps[0:1, n * 512:(n + 1) * 512],
                                         lhsT=ones_sb[:, 0:1],
                                         rhs=sq[:, n * 512:(n + 1) * 512],
                                         start=(t == 0), stop=(t == CT - 1))
                row = pab.tile([1, L], F32R, tag=f"row{tag}",
                               name=f"row{tag}")
                rsqrt_row(r_ps, C, row, tag)
                bc_ps = psum.tile([128, L], F32, tag=big_tag, bufs=2,
                                  name=f"bc_{tag}")
                for n in range(NT):
                    nc.tensor.matmul(out=bc_ps[:, n * 512:(n + 1) * 512],
                                     lhsT=ones_sb[0:1, :],
                                     rhs=row[0:1, n * 512:(n + 1) * 512],
                                     start=True, stop=True)
                outs = []
                for t in range(CT):
                    nt = pab.tile([128, L], F32R, tag=f"n{tag}{t}",
                                  name=f"n{tag}{t}")
                    nc.vector.tensor_mul(nt[:, :],
                                         src_tiles[t][:, :].bitcast(F32),
                                         bc_ps[:, :])
                    outs.append(nt)
                return outs

            xn = rms_normalize(x_sb, "x", "sim2")
            cn = rms_normalize(ct_sb, "c", "sim2")

            # ---- projections ----
            alt_ctr = [0]

            def alt_psum(w):
                alt_ctr[0] += 1
                return psum.tile([128, w], F32, tag="sim2", bufs=2,
                                 name=f"altps{alt_ctr[0]}")

            for m in range(CT):  # q & k: out [hidden-tile, token]
                for dst, w_t, src in ((q_sb, wq_sb, xn), (k_sb, wk_sb, cn)):
                    mm_ps = alt_psum(L)
                    for n in range(NT):
                        for t in range(CT):
                            nc.tensor.matmul(
                                out=mm_ps[:, n * 512:(n + 1) * 512],
                                lhsT=w_t[t][:, m * 128:(m + 1) * 128],
                                rhs=src[t][:, n * 512:(n + 1) * 512],
                                start=(t == 0), stop=(t == CT - 1))
                    nc.vector.tensor_copy(dst[m][:, :], mm_ps[:, :])

            for j in range(JT):  # vT: out [j-tile, hidden], augmented layout
                mm_ps = alt_psum(HID)
                for t in range(CT):
                    nc.tensor.matmul(out=mm_ps[:, :],
                                     lhsT=cn[t][:, j * 128:(j + 1) * 128],
                                     rhs=wv_sb[t][:, :],
                                     start=(t == 0), stop=(t == CT - 1))
                vh = vT_sb[j][:, :].rearrange("p (h c) -> p h c", h=HEADS)
                nc.vector.tensor_copy(
                    vh[:, :, 0:HD],
                    mm_ps[:, :].rearrange("p (h c) -> p h c", h=HEADS))
                nc.gpsimd.dma_start(out=vh[:, :, HD:VW],
                                    in_=one_d[:, :].to_broadcast((128, HEADS, 1)))

        # =========== stage C: attention ===========
        with ExitStack() as sc:
            pexp = sc.enter_context(tc.tile_pool(name="exp", bufs=4))
            pou = sc.enter_context(tc.tile_pool(name="ou", bufs=HEADS))
            psmall = sc.enter_context(tc.tile_pool(name="small", bufs=1))

            ssum_all = psmall.tile([HEADS, L], F32)
            ou_tiles = []

            # flat chunk stream: chunk c = (head h = c//16, j = (c%16)//2,
            # n-half = c%2); sim tiles cover 3 chunks (3 psum banks, bufs=2)
            # so ACT exp of tile t overlaps PE sim-fill of tile t+1; PV
            # emission lags one tile behind the exps.
            TOTAL_CH = HEADS * JT * NT  # 128
            CPT = 3
            n_ctiles = (TOTAL_CH + CPT - 1) // CPT
            ex_tiles = []
            tile_chunks = []
            ou_cur = {}

            def emit_sims(tix):
                c0 = tix * CPT
                chunks = list(range(c0, min(c0 + CPT, TOTAL_CH)))
                w = len(chunks) * 512
                sim_ps = psum.tile([128, w], F32, tag="sim2",
                                   name=f"simps{tix}", bufs=2)
                for ci, c in enumerate(chunks):
                    h, r = c // 16, c % 16
                    j, n = r // 2, r % 2
                    mt, po = h // 2, (h % 2) * 64
                    nc.tensor.matmul(
                        out=sim_ps[:, ci * 512:(ci + 1) * 512],
                        lhsT=k_sb[mt][po:po + HD, j * 128:(j + 1) * 128],
                        rhs=q_sb[mt][po:po + HD, n * 512:(n + 1) * 512],
                        start=True, stop=True)
                ex = pexp.tile([128, w], F32R, tag="exp", name=f"ex{tix}")
                nc.scalar.activation(out=ex[:, :], in_=sim_ps[:, :],
                                     func=AF.Exp, bias=0.0, scale=1.0)
                ex_tiles.append(ex)
                tile_chunks.append(chunks)

            def emit_pvs(tix):
                for ci, c in enumerate(tile_chunks[tix]):
                    h, r = c // 16, c % 16
                    j, n = r // 2, r % 2
                    if r == 0:
                        ou_cur[h] = psum.tile([VW, L], F32, tag="ou",
                                              name=f"oups{h}")
                    nc.tensor.matmul(
                        out=ou_cur[h][:, n * 512:(n + 1) * 512],
                        lhsT=vT_sb[j][:, h * VW:(h + 1) * VW],
                        rhs=ex_tiles[tix][:, ci * 512:(ci + 1) * 512],
                        start=(j == 0), stop=(j == JT - 1))
                    if r == 15:
                        ou_sb = pou.tile([VW, L], F32, tag="ousb",
                                         name=f"ousb{h}")
                        nc.vector.tensor_copy(ou_sb[:, :], ou_cur[h][:, :])
                        nc.sync.dma_start(out=ssum_all[h:h + 1, :],
                                          in_=ou_sb[HD:VW, :])
                        ou_tiles.append(ou_sb)

            for tix in range(n_ctiles):
                emit_sims(tix)
                if tix >= 1:
                    emit_pvs(tix - 1)
            emit_pvs(n_ctiles - 1)

            # reciprocal of all denominators at once: 1/s = exp(-ln(s))
            ln_s = psmall.tile([HEADS, L], F32)
            nc.scalar.activation(out=ln_s[:, :], in_=ssum_all[:, :], func=AF.Ln,
                                 bias=0.0, scale=1.0)
            rec_all = psmall.tile([HEADS, L], F32R)
            nc.scalar.activation(out=rec_all[:, :], in_=ln_s[:, :], func=AF.Exp,
                                 bias=0.0, scale=-1.0)

            for mt in range(CT):
                for n in range(NT):
                    ns = slice(n * 512, (n + 1) * 512)
                    rec_ps = psum.tile([128, 512], F32, tag="sim2", bufs=2)
                    nc.tensor.matmul(out=rec_ps[:, :],
                                     lhsT=sel_sb[0:8, mt * 128:(mt + 1) * 128],
                                     rhs=rec_all[0:8, ns],
                                     start=True, stop=True)
                    nc.vector.tensor_mul(
                        ao_sb[mt][0:HD, ns],
                        ou_tiles[2 * mt][0:HD, ns], rec_ps[0:64, :])
                    nc.vector.tensor_mul(
                        ao_sb[mt][HD:128, ns],
                        ou_tiles[2 * mt + 1][0:HD, ns], rec_ps[64:128, :])

        # =========== stage D: output projection + norm + residual ===========
        with ExitStack() as sd:
            pd = sd.enter_context(tc.tile_pool(name="d", bufs=1))
            y_sb, ysq = [], []
            for m in range(CT):
                y_ps = psum.tile([128, L], F32, tag="sim2", bufs=2,
                                 name=f"yps{m}")
                for n in range(NT):
                    for t in range(CT):
                        nc.tensor.matmul(
                            out=y_ps[:, n * 512:(n + 1) * 512],
                            lhsT=wo_sb[t][:, m * 128:(m + 1) * 128],
                            rhs=ao_sb[t][:, n * 512:(n + 1) * 512],
                            start=(t == 0), stop=(t == CT - 1))
                yt = pd.tile([128, L], F32, tag=f"y{m}")
                nc.vector.tensor_scalar_add(yt[:, :], y_ps[:, :], bo_sb[m][:, :])
                y_sb.append(yt)
                s = pd.tile([128, L], F32R, tag=f"ysq{m}")
                nc.vector.tensor_mul(s[:, :], yt[:, :], yt[:, :])
                ysq.append(s)

            r3_ps = psum.tile([1, L], F32, tag="ou", name="r3ps")
            for n in range(NT):
                for t in range(CT):
                    nc.tensor.matmul(out=r3_ps[0:1, n * 512:(n + 1) * 512],
                                     lhsT=ones_sb[:, 0:1],
                                     rhs=ysq[t][:, n * 512:(n + 1) * 512],
                                     start=(t == 0), stop=(t == CT - 1))
            r3_row = pd.tile([1, L], F32R, tag="r3row")
            rsqrt_row(r3_ps, C, r3_row, "r3")
            bc3_ps = psum.tile([128, L], F32, tag="sim2", bufs=2, name="bc3ps")
            for n in range(NT):
                nc.tensor.matmul(out=bc3_ps[:, n * 512:(n + 1) * 512],
                                 lhsT=ones_sb[0:1, :],
                                 rhs=r3_row[0:1, n * 512:(n + 1) * 512],
                                 start=True, stop=True)
            for m in range(CT):
                tmp = pd.tile([128, L], F32, tag=f"tmp{m}")
                nc.vector.scalar_tensor_tensor(
                    out=tmp[:, :], in0=y_sb[m][:, :], scalar=g2_sb[m][:, :],
                    in1=bc3_ps[:, :], op0=mybir.AluOpType.mult,
                    op1=mybir.AluOpType.mult)
                fin = pd.tile([128, L], F32, tag=f"fin{m}")
                nc.vector.tensor_add(fin[:, :], tmp[:, :],
                                     x_sb[m][:, :].bitcast(F32))
                nc.sync.dma_start(out=y_d[m * 128:(m + 1) * 128, :],
                                  in_=fin[:, :])

    nc.compile()
    return nc


_NC_CACHE = {}


def _get_nc():
    if "nc" not in _NC_CACHE:
        _NC_CACHE["nc"] = build()
    return _NC_CACHE["nc"]


def kernel(x, context, Wq, Wkv, Wo, bo, g, g2):
    x = np.asarray(x, dtype=np.float32)
    context = np.asarray(context, dtype=np.float32)
    Wq = np.asarray(Wq, dtype=np.float32)
    Wkv = np.asarray(Wkv, dtype=np.float32)
    Wo = np.asarray(Wo, dtype=np.float32)
    bo = np.asarray(bo, dtype=np.float32)
    g = np.asarray(g, dtype=np.float32)
    g2 = np.asarray(g2, dtype=np.float32)

    scale = HD ** -0.5
    wq_h = np.ascontiguousarray((Wq * g[None, :] * scale).T)  # [C, HID]
    wk_h = np.ascontiguousarray((Wkv[:HID] * g[None, :]).T)   # [C, HID]
    wv_h = np.ascontiguousarray((Wkv[HID:] * g[None, :]).T)   # [C, HID]
    wo_h = np.ascontiguousarray(Wo.T)                         # [HID, C]
    bog2 = np.ascontiguousarray(np.stack([bo, g2], axis=1))   # [C, 2]
    ones = np.ones((128, 128), dtype=np.float32)
    one = np.ones((1, 1), dtype=np.float32)
    sel = np.zeros((8, 512), dtype=np.float32)
    for mt in range(4):
        sel[2 * mt, mt * 128:mt * 128 + 64] = 1.0
        sel[2 * mt + 1, mt * 128 + 64:mt * 128 + 128] = 1.0

    nc = _get_nc()
    global _last_in_maps
    in_maps = []
    for i in range(NCORES):
        in_maps.append({
            "x": np.ascontiguousarray(x[i].reshape(C, L)),
            "ctxT": np.ascontiguousarray(context[i].T),
            "wq": wq_h, "wk": wk_h, "wv": wv_h, "wo": wo_h,
            "ones": ones, "one": one, "bog2": bog2, "sel": sel,
        })
    _last_in_maps = in_maps
    res = run_bass_kernel_spmd(nc, in_maps, list(range(NCORES)))
    out = np.stack([res.results[i]["y_out"].reshape(C, H, W)
                    for i in range(NCORES)])
    return out.astype(np.float32)


_last_in_maps = None



# revision 25
# speedup vs baseline: 1.0369x; 1.0369x over previous
"""Trainium2 Bass kernel for nn_CrossAttention (B=8, C=512, H=W=32, Lc=1024,
8 heads x 64 dim).

Sharding: data-parallel over batch B across the 8 NeuronCores (1 image/core,
no collectives). Feature-on-partitions layout; all matmuls contract over SBUF
partitions.

Optimizations over the v1 kernel (271.8us):
  - bf16 inputs (host-cast): 4MB instead of 8MB HBM per core.
  - Input DMAs spread across both HWDGE rings (sync + scalar) + SWDGE; no
    4-byte-element descriptor DMAs.
  - sim matmuls row-tiled: the two K=64 heads of a pair run concurrently in
    separate PE row-groups (col tiling is not supported by the compiler).
  - PV uses the ones-augmented vT (65 cols/head) so softmax denominators
    fall out of the PV matmul for free.
  - ACT does ONLY Exp (one table load). RMS rsqrt rows are computed in
    transposed [128, n] form via per-chunk stats matmuls (N=2), then a
    Quake-seed + 2x Newton rsqrt on DVE. Softmax reciprocal via
    nc.vector.reciprocal.
  - Per-token RMS factor rc of the context is applied for free as the
    per-partition ACT scale of the exp, and folded into vT at copy time.
  - exp runs over [128, 1024] PSUM pair-tiles (two heads per ACT op).
  - PE warmed up with dummy matmuls during the input-DMA window (HAM clock).
  - n-major stage C; stage D of n=0 hides under the ACT window of n=1.
  - software-pipelined emission: per-engine FIFOs never stall on bank WARs.
"""

import numpy as np
import ml_dtypes
from contextlib import ExitStack

import concourse.bass as bass
from concourse import bacc
import concourse.mybir as mybir
import concourse.tile as tile
from concourse.bass_utils import run_bass_kernel_spmd

F32 = mybir.dt.float32
F32R = mybir.dt.float32r
BF16 = mybir.dt.bfloat16
I32 = mybir.dt.int32
AF = mybir.ActivationFunctionType
OP = mybir.AluOpType

B, C, H, W = 8, 512, 32, 32
L = H * W  # 1024 query pixels
LC = 1024  # context tokens
HEADS, HD = 8, 64
VW = HD + 1  # 65: v columns + ones column (emits softmax denominator)
HID = HEADS * HD  # 512
EPS = 1e-6
NCORES = 8
CT = C // 128  # 4 c-tiles
JT = LC // 128  # 8 j-tiles

MAGIC = 0x5F3759DF


def build():
    nc = bacc.Bacc("TRN2", target_bir_lowering=False, debug=False,
                   num_devices=NCORES)

    x_d = nc.dram_tensor("x", [C, L], BF16, kind="ExternalInput")
    ct_d = nc.dram_tensor("ctxT", [C, LC], BF16, kind="ExternalInput")
    wq_d = nc.dram_tensor("wq", [C, HID], BF16, kind="ExternalInput")
    wk_d = nc.dram_tensor("wk", [C, HID], BF16, kind="ExternalInput")
    wv_d = nc.dram_tensor("wv", [C, HID], BF16, kind="ExternalInput")
    wo_d = nc.dram_tensor("wo", [HID, C], BF16, kind="ExternalInput")
    ones_d = nc.dram_tensor("ones", [128, 128], F32R, kind="ExternalInput")
    ident_d = nc.dram_tensor("ident", [128, 128], F32R, kind="ExternalInput")
    bog2_d = nc.dram_tensor("bog2T", [2, C], F32R, kind="ExternalInput")
    y_d = nc.dram_tensor("y_out", [C, L], F32, kind="ExternalOutput")

    with tile.TileContext(nc) as tc, ExitStack() as top:
        pc = top.enter_context(tc.tile_pool(name="main", bufs=1))
        psum = top.enter_context(tc.tile_pool(name="ps", bufs=1, space="PSUM"))

        # ---------------- input DMAs (spread across rings) ----------------
        ct_sb, x_sb = [], []
        for t in range(CT):
            ctt = pc.tile([128, LC], BF16, tag=f"ct{t}")
            nc.sync.dma_start(out=ctt, in_=ct_d[t * 128:(t + 1) * 128, :])
            ct_sb.append(ctt)
        for t in range(CT):
            xt = pc.tile([128, L], BF16, tag=f"x{t}")
            nc.sync.dma_start(out=xt, in_=x_d[t * 128:(t + 1) * 128, :])
            x_sb.append(xt)
        wk_sb, wq_sb, wv_sb = [], [], []
        for name, lst, dram in (("wk", wk_sb, wk_d), ("wq", wq_sb, wq_d),
                                ("wv", wv_sb, wv_d)):
            for t in range(CT):
                wt = pc.tile([128, HID], BF16, tag=f"{name}{t}")
                nc.scalar.dma_start(out=wt, in_=dram[t * 128:(t + 1) * 128, :])
                lst.append(wt)
        ones_sb = pc.tile([128, 128], F32R, tag="ones")
        nc.gpsimd.dma_start(out=ones_sb, in_=ones_d[:, :])
        ident_sb = pc.tile([128, 128], F32R, tag="ident")
        nc.gpsimd.dma_start(out=ident_sb, in_=ident_d[:, :])
        bog2_sb = pc.tile([2, C], F32R, tag="bog2")
        nc.gpsimd.dma_start(out=bog2_sb, in_=bog2_d[:, :])
        wo_sb = []
        for t in range(CT):
            wt = pc.tile([128, C], BF16, tag=f"wo{t}")
            nc.gpsimd.dma_start(out=wt, in_=wo_d[t * 128:(t + 1) * 128, :])
            wo_sb.append(wt)

        # ---------------- PE warmup (runs during DMA wait) ----------------
        warm_sb = pc.tile([128, 512], F32, tag="warm")
        nc.vector.memset(warm_sb, 1.0)
        warm_ps = psum.tile([128, 512], F32, tag="spare", name="warmps",
                            bufs=2)
        for i in range(10):
            nc.tensor.matmul(out=warm_ps[:, :],
                             lhsT=warm_sb[:, 0:128].bitcast(F32R),
                             rhs=warm_sb[:, :].bitcast(F32R),
                             start=True, stop=True)
        warm_ex = pc.tile([1, 8], F32R, tag="warmex")
        nc.scalar.activation(out=warm_ex[:, :], in_=warm_sb[0:1, 0:8],
                             func=AF.Exp, bias=0.0, scale=0.0)

        # ---------------- squares + transposed stats -----------------------
        # ssq cols (pairs, col 2c used): 0:16 x-pixel chunks, 16:32 ctx
        ssq_ps = psum.tile([128, 512], F32, tag="sim", name="ssqps", bufs=2)
        sq_x, sq_c = [], []
        for t in range(CT):
            s = pc.tile([128, L], F32R, tag="sq", name=f"sqx{t}", bufs=4)
            nc.gpsimd.tensor_mul(s[:, :], x_sb[t][:, :], x_sb[t][:, :])
            sq_x.append(s)
        for t in range(CT):
            s = pc.tile([128, LC], F32R, tag="sq", name=f"sqc{t}", bufs=4)
            nc.gpsimd.tensor_mul(s[:, :], ct_sb[t][:, :], ct_sb[t][:, :])
            sq_c.append(s)
        for c in range(8):
            for t in range(CT):
                nc.tensor.matmul(out=ssq_ps[:, 2 * c:2 * c + 2],
                                 lhsT=sq_x[t][:, c * 128:(c + 1) * 128],
                                 rhs=ones_sb[:, 0:2],
                                 start=(t == 0), stop=(t == CT - 1))
        for c in range(8):
            for t in range(CT):
                nc.tensor.matmul(out=ssq_ps[:, 16 + 2 * c:18 + 2 * c],
                                 lhsT=sq_c[t][:, c * 128:(c + 1) * 128],
                                 rhs=ones_sb[:, 0:2],
                                 start=(t == 0), stop=(t == CT - 1))

        # Quake rsqrt on DVE: dst = (src/nfeat + eps)^-0.5
        kmagic = pc.tile([128, 32], I32, tag="kmagic")
        nc.vector.memset(kmagic, MAGIC)

        def dve_rsqrt(dst, src_ps, ncols, nfeat, scratch_tag):
            m = pc.tile([128, ncols], F32, tag=f"{scratch_tag}m")
            nc.vector.tensor_scalar(out=m[:, :], in0=src_ps[:, 0:ncols],
                                    scalar1=1.0 / nfeat, scalar2=EPS,
                                    op0=OP.mult, op1=OP.add)
            m2 = pc.tile([128, ncols], F32, tag=f"{scratch_tag}m2")
            nc.vector.tensor_scalar(out=m2[:, :], in0=src_ps[:, 0:ncols],
                                    scalar1=0.5 / nfeat, scalar2=0.5 * EPS,
                                    op0=OP.mult, op1=OP.add)
            sh = pc.tile([128, ncols], I32, tag=f"{scratch_tag}sh")
            nc.vector.tensor_scalar(out=sh[:, :],
                                    in0=m[:, :].bitcast(I32),
                                    scalar1=1, scalar2=0,
                                    op0=OP.logical_shift_right,
                                    op1=OP.logical_shift_right)
            y0 = pc.tile([128, ncols], F32, tag=f"{scratch_tag}y0")
            nc.vector.scalar_tensor_tensor(
                out=y0[:, :].bitcast(I32), in0=kmagic[:, 0:ncols], scalar=0,
                in1=sh[:, :], op0=OP.add, op1=OP.subtract)
            # 2 Newton iters, negated form (signs cancel):
            # y' = (m2*y^2 - 1.5) * y
            t1 = pc.tile([128, ncols], F32, tag=f"{scratch_tag}t1")
            y1 = pc.tile([128, ncols], F32, tag=f"{scratch_tag}y1")
            nc.vector.tensor_mul(t1[:, :], y0[:, :], y0[:, :])
            nc.vector.tensor_mul(t1[:, :], t1[:, :], m2[:, :])
            nc.vector.scalar_tensor_tensor(
                out=y1[:, :], in0=t1[:, :], scalar=1.5, in1=y0[:, :],
                op0=OP.subtract, op1=OP.mult)
            nc.vector.tensor_mul(t1[:, :], y1[:, :], y1[:, :])
            nc.vector.tensor_mul(t1[:, :], t1[:, :], m2[:, :])
            nc.vector.scalar_tensor_tensor(
                out=dst[:, :], in0=t1[:, :], scalar=1.5, in1=y1[:, :],
                op0=OP.subtract, op1=OP.mult)

        # ---------------- projection machinery -----------------------------
        q_sb = [pc.tile([128, L], F32R, tag=f"q{m}", name=f"q{m}")
                for m in range(CT)]
        k_sb = [pc.tile([128, LC], F32R, tag=f"k{m}", name=f"k{m}")
                for m in range(CT)]
        vT_sb = []
        for j in range(JT):
            vt = pc.tile([128, HEADS * VW], F32R, tag=f"vT{j}", name=f"vT{j}")
            vh = vt[:, :].rearrange("p (h c) -> p h c", h=HEADS)
            nc.vector.memset(vh[:, :, HD:VW].bitcast(F32), 1.0)
            vT_sb.append(vt)
        ao_sb = [pc.tile([128, L], BF16, tag=f"ao{m}", name=f"ao{m}")
                 for m in range(CT)]
        rsq_xc = pc.tile([128, 32], F32, tag="rsqxc")
        bcx_sb = pc.tile([128, L], F32R, tag="bcx")

        def proj_q(m, n, ptag):
            ns = slice(n * 512, (n + 1) * 512)
            ps = psum.tile([128, 512], F32, tag=ptag, name=f"qp{m}{n}",
                           bufs=2)
            for t in range(CT):
                nc.tensor.matmul(out=ps[:, :],
                                 lhsT=wq_sb[t][:, m * 128:(m + 1) * 128],
                                 rhs=x_sb[t][:, ns],
                                 start=(t == 0), stop=(t == CT - 1))
            nc.vector.tensor_mul(q_sb[m][:, ns], ps[:, :],
                                 bcx_sb[:, ns].bitcast(F32))

        def proj_k(m, h, ptag):
            hs = slice(h * 512, (h + 1) * 512)
            ps = psum.tile([128, 512], F32, tag=ptag, name=f"kp{m}{h}",
                           bufs=2)
            for t in range(CT):
                nc.tensor.matmul(out=ps[:, :],
                                 lhsT=wk_sb[t][:, m * 128:(m + 1) * 128],
                                 rhs=ct_sb[t][:, hs],
                                 start=(t == 0), stop=(t == CT - 1))
            nc.vector.tensor_copy(k_sb[m][:, hs], ps[:, :])

        def proj_v(j, ptag):
            ps = psum.tile([128, HID], F32, tag=ptag, name=f"vp{j}",
                           bufs=2)
            for t in range(CT):
                nc.tensor.matmul(out=ps[:, :],
                                 lhsT=ct_sb[t][:, j * 128:(j + 1) * 128],
                                 rhs=wv_sb[t][:, :],
                                 start=(t == 0), stop=(t == CT - 1))
            vh = vT_sb[j][:, :].rearrange("p (h c) -> p h c", h=HEADS)
            # fold per-token rms factor rc into v
            nc.vector.tensor_scalar_mul(
                vh[:, :, 0:HD],
                ps[:, :].rearrange("p (h c) -> p h c", h=HEADS),
                rsq_xc[:, 16 + 2 * j:17 + 2 * j])

        # k projections first on the PE queue (only need ctx + wk DMAs)
        proj_k(0, 0, "spare")
        proj_k(0, 1, "ou")
        proj_k(1, 0, "spare")
        proj_k(1, 1, "ou")

        # rsq_xc cols (2c): 0:16 pixels rxT, 16:32 tokens rcT
        dve_rsqrt(rsq_xc, ssq_ps, 32, C, "rs")

        # bc_rx [128, L]: bc_rx[p, i] = rx[i] via diag trick
        diag_t = [pc.tile([128, 128], F32R, tag="diag", name=f"dg{c}", bufs=2)
                  for c in range(8)]
        bcx_ps = psum.tile([128, L], F32, tag="sim", name="bcxps", bufs=2)
        for c in range(8):
            nc.vector.tensor_scalar_mul(diag_t[c][:, :],
                                        ident_sb[:, :].bitcast(F32),
                                        rsq_xc[:, 2 * c:2 * c + 1])
            nc.tensor.matmul(out=bcx_ps[:, c * 128:(c + 1) * 128],
                             lhsT=ones_sb[:, :], rhs=diag_t[c][:, :],
                             start=True, stop=True)
        nc.vector.tensor_copy(bcx_sb[:, :], bcx_ps[:, :])

        # bog2 "transpose": [2, C] row layout -> [128, 2] per c-tile
        bo_sb, g2_sb = [], []
        for t in range(CT):
            bps = psum.tile([128, 512], F32, tag="ou", name=f"bog{t}", bufs=2)
            nc.tensor.matmul(out=bps[:, 0:2],
                             lhsT=bog2_sb[:, t * 128:(t + 1) * 128],
                             rhs=ident_sb[0:2, 0:2],
                             start=True, stop=True)
            bg = pc.tile([128, 2], F32, tag=f"bog2s{t}")
            nc.vector.tensor_copy(bg[:, :], bps[:, 0:2])
            bo_sb.append(bg[:, 0:1])
            g2_sb.append(bg[:, 1:2])

        # rest of the pre-attention projections
        proj_q(0, 0, "spare")
        proj_q(1, 0, "ou")
        proj_v(0, "spare")
        proj_v(1, "ou")
        proj_v(2, "spare")
        proj_v(3, "ou")
        proj_v(4, "spare")
        proj_v(5, "ou")

        # deferred projection work, drained into stage-C PE slack
        filler = []
        filler.append(lambda: proj_v(6, "spare"))
        filler.append(lambda: proj_v(7, "spare"))
        filler.append(lambda: proj_k(2, 0, "spare"))
        filler.append(lambda: proj_k(2, 1, "spare"))
        filler.append(lambda: proj_k(3, 0, "spare"))
        filler.append(lambda: proj_k(3, 1, "spare"))
        filler.append(lambda: proj_q(2, 0, "spare"))
        filler.append(lambda: proj_q(3, 0, "spare"))
        for m in range(CT):
            filler.append(lambda m=m: proj_q(m, 1, "spare"))

        # ---------------- stage D (emitted later, per n) --------------------
        xf32 = []
        for t in range(CT):
            xf = pc.tile([128, L], F32, tag=f"xf{t}", name=f"xf{t}")
            nc.gpsimd.tensor_copy(xf[:, :], x_sb[t][:, :])
            xf32.append(xf)
        ybig = pc.tile([128, 4 * L], F32, tag="ybig")
        ysq_t = [pc.tile([128, 512], F32R, tag=f"ysq{m}", name=f"ysq{m}")
                 for m in range(CT)]

        def stage_d(n):
            ns = slice(n * 512, (n + 1) * 512)
            ops = []
            for m in range(CT):
                def dproj(m=m):
                    ps = psum.tile([128, 512], F32, tag="spare",
                                   name=f"yp{m}{n}", bufs=2)
                    for t in range(CT):
                        nc.tensor.matmul(
                            out=ps[:, :],
                            lhsT=wo_sb[t][:, m * 128:(m + 1) * 128],
                            rhs=ao_sb[t][:, ns],
                            start=(t == 0), stop=(t == CT - 1))
                    ysl = ybig[:, m * L + n * 512: m * L + (n + 1) * 512]
                    nc.vector.tensor_scalar_add(ysl, ps[:, :], bo_sb[m])
                    nc.vector.tensor_mul(ysq_t[m][:, :], ysl, ysl)
                ops.append(dproj)

            def dstat():
                ssy = psum.tile([128, 512], F32, tag="sim", bufs=2,
                                name=f"ssy{n}")
                for c in range(4):
                    for m in range(CT):
                        nc.tensor.matmul(
                            out=ssy[:, 2 * c:2 * c + 2],
                            lhsT=ysq_t[m][:, c * 128:(c + 1) * 128],
                            rhs=ones_sb[:, 0:2],
                            start=(m == 0), stop=(m == CT - 1))
                ry = pc.tile([128, 8], F32, tag=f"ry{n}")
                dve_rsqrt(ry, ssy, 8, C, f"ry{n}")
                bcy = psum.tile([128, 512], F32, tag="spare", name=f"bcy{n}",
                                bufs=2)
                for c in range(4):
                    dg = pc.tile([128, 128], F32R, tag="diag",
                                 name=f"dgy{n}{c}", bufs=2)
                    nc.vector.tensor_scalar_mul(dg[:, :],
                                                ident_sb[:, :].bitcast(F32),
                                                ry[:, 2 * c:2 * c + 1])
                    nc.tensor.matmul(out=bcy[:, c * 128:(c + 1) * 128],
                                     lhsT=ones_sb[:, :], rhs=dg[:, :],
                                     start=True, stop=True)
                for m in range(CT):
                    ysl = ybig[:, m * L + n * 512: m * L + (n + 1) * 512]
                    tmp = pc.tile([128, 512], F32, tag="fintmp",
                                  name=f"ft{n}{m}", bufs=2)
                    nc.vector.scalar_tensor_tensor(
                        out=tmp[:, :], in0=ysl, scalar=g2_sb[m],
                        in1=bcy[:, :], op0=OP.mult, op1=OP.mult)
                    nc.vector.tensor_add(ysl, tmp[:, :], xf32[m][:, ns])
                    nc.sync.dma_start(
                        out=y_d[m * 128:(m + 1) * 128, ns], in_=ysl)
            ops.append(dstat)
            return ops

        # ---------------- stage C: attention -------------------------------
        pexp = top.enter_context(tc.tile_pool(name="exp", bufs=1))

        steps = [(n, p, j) for n in range(2) for p in range(4)
                 for j in range(JT)]

        sim_slots = {}
        ex_slots = {}

        def emit_sims(step):
            n, p, j = step
            ns = slice(n * 512, (n + 1) * 512)
            js = slice(j * 128, (j + 1) * 128)
            sl = psum.tile([128, 1024], F32, tag="sim", bufs=2,
                           name=f"sim{n}{p}{j}")
            nc.tensor.matmul(out=sl[:, 0:512],
                             lhsT=k_sb[p][0:64, js],
                             rhs=q_sb[p][0:64, ns],
                             start=True, stop=True)
            nc.tensor.matmul(out=sl[:, 512:1024],
                             lhsT=k_sb[p][64:128, js],
                             rhs=q_sb[p][64:128, ns],
                             start=True, stop=True)
            sim_slots[step] = sl

        def emit_exps(step):
            n, p, j = step
            ex = pexp.tile([128, 1024], F32R, tag="ex", bufs=4,
                           name=f"ex{n}{p}{j}")
            nc.scalar.activation(out=ex[:, :], in_=sim_slots[step][:, :],
                                 func=AF.Exp, bias=0.0,
                                 scale=rsq_xc[:, 16 + 2 * j:17 + 2 * j])
            ex_slots[step] = ex

        ou_cur = {}

        def emit_pv(step):
            n, p, j = step
            if j == 0:
                ou_cur[0] = psum.tile([128, 512], F32, tag="ou", bufs=2,
                                      name=f"ou{n}{p}0")
                ou_cur[1] = psum.tile([128, 512], F32, tag="ou", bufs=2,
                                      name=f"ou{n}{p}1")
            ex = ex_slots[step]
            for hi in range(2):
                h = 2 * p + hi  # global head
                nc.tensor.matmul(
                    out=ou_cur[hi][0:VW, :],
                    lhsT=vT_sb[j][:, h * VW:(h + 1) * VW],
                    rhs=ex[:, hi * 512:(hi + 1) * 512],
                    start=(j == 0), stop=(j == JT - 1))

        def emit_pair_end(step):
            n, p, j = step
            ns = slice(n * 512, (n + 1) * 512)
            for hi in range(2):
                rec = pc.tile([1, 512], F32R, tag="rec",
                              name=f"rc{n}{p}{hi}", bufs=4)
                with nc.allow_low_precision(reason="softmax denom recip"):
                    nc.vector.reciprocal(rec[:, :], ou_cur[hi][HD:VW, :])
                bcr = psum.tile([128, 512], F32, tag="sim", bufs=2,
                                name=f"bcr{n}{p}{hi}")
                nc.tensor.matmul(out=bcr[0:HD, :],
                                 lhsT=ones_sb[0:1, 0:HD],
                                 rhs=rec[:, :],
                                 start=True, stop=True)
                bcs = pc.tile([64, 512], F32, tag="bcs",
                              name=f"bcs{n}{p}{hi}", bufs=2)
                nc.vector.tensor_copy(bcs[:, :], bcr[0:HD, :])
                nc.vector.tensor_mul(
                    ao_sb[p][hi * HD:(hi + 1) * HD, ns],
                    ou_cur[hi][0:HD, :], bcs[:, :])

        # ---- emission with software pipelining ----
        d_ops = []
        emit_sims(steps[0])
        for si, step in enumerate(steps):
            n, p, j = step
            emit_exps(step)
            if si + 1 < len(steps):
                emit_sims(steps[si + 1])
            emit_pv(step)
            if j == JT - 1:
                emit_pair_end(step)
                if (n, p) == (0, 3):
                    d_ops = stage_d(0)
                elif (n, p) == (1, 3):
                    for op in stage_d(1):
                        op()
            # drain deferred work into PE slack: one PSUM-serial group
            # every other step so the PE FIFO never stalls on a bank WAR
            if si % 2 == 1:
                if filler:
                    filler.pop(0)()
                elif d_ops and si >= 34:
                    d_ops.pop(0)()
        for op in d_ops:
            op()

    nc.compile()
    return nc


_NC_CACHE = {}


def _get_nc():
    if "nc" not in _NC_CACHE:
        _NC_CACHE["nc"] = build()
    return _NC_CACHE["nc"]


def kernel(x, context, Wq, Wkv, Wo, bo, g, g2):
    x = np.asarray(x, dtype=np.float32)
    context = np.asarray(context, dtype=np.float32)
    Wq = np.asarray(Wq, dtype=np.float32)
    Wkv = np.asarray(Wkv, dtype=np.float32)
    Wo = np.asarray(Wo, dtype=np.float32)
    bo = np.asarray(bo, dtype=np.float32)
    g = np.asarray(g, dtype=np.float32)
    g2 = np.asarray(g2, dtype=np.float32)

    bf = ml_dtypes.bfloat16
    scale = HD ** -0.5
    wq_h = np.ascontiguousarray((Wq * g[None, :] * scale).T).astype(bf)
    wk_h = np.ascontiguousarray((Wkv[:HID] * g[None, :]).T).astype(bf)
    wv_h = np.ascontiguousarray((Wkv[HID:] * g[None, :]).T).astype(bf)
    wo_h = np.ascontiguousarray(Wo.T).astype(bf)
    bog2T = np.ascontiguousarray(np.stack([bo, g2], axis=0))  # [2, C]
    ones = np.ones((128, 128), dtype=np.float32)
    ident = np.eye(128, dtype=np.float32)

    nc = _get_nc()
    global _last_in_maps
    in_maps = []
    for i in range(NCORES):
        in_maps.append({
            "x": np.ascontiguousarray(x[i].reshape(C, L)).astype(bf),
            "ctxT": np.ascontiguousarray(context[i].T).astype(bf),
            "wq": wq_h, "wk": wk_h, "wv": wv_h, "wo": wo_h,
            "ones": ones, "ident": ident, "bog2T": bog2T,
        })
    _last_in_maps = in_maps
    res = run_bass_kernel_spmd(nc, in_maps, list(range(NCORES)))
    out = np.stack([res.results[i]["y_out"].reshape(C, H, W)
                    for i in range(NCORES)])
    return out.astype(np.float32)


_last_in_maps = None


# revision 26
# speedup vs baseline: 1.4287x; 1.3779x over previous
"""Trainium2 Bass kernel for nn_CrossAttention (B=8, C=512, H=W=32, Lc=1024,
8 heads x 64 dim).

Sharding: data-parallel over batch B across the 8 NeuronCores (1 image/core,
no collectives). Feature-on-partitions layout; all matmuls contract over SBUF
partitions.

Optimizations over the v1 kernel (271.8us):
  - bf16 inputs (host-cast): 4MB instead of 8MB HBM per core.
  - Input DMAs spread across both HWDGE rings (sync + scalar) + SWDGE; no
    4-byte-element descriptor DMAs.
  - sim matmuls row-tiled: the two K=64 heads of a pair run concurrently in
    separate PE row-groups (col tiling is not supported by the compiler).
  - PV uses the ones-augmented vT (65 cols/head) so softmax denominators
    fall out of the PV matmul for free.
  - ACT does ONLY Exp (one table load). RMS rsqrt rows are computed in
    transposed [128, n] form via per-chunk stats matmuls (N=2), then a
    Quake-seed + 2x Newton rsqrt on DVE. Softmax reciprocal via
    nc.vector.reciprocal.
  - Per-token RMS factor rc of the context is applied for free as the
    per-partition ACT scale of the exp, and folded into vT at copy time.
  - exp runs over [128, 1024] PSUM pair-tiles (two heads per ACT op).
  - PE warmed up with dummy matmuls during the input-DMA window (HAM clock).
  - n-major stage C; stage D of n=0 hides under the ACT window of n=1.
  - software-pipelined emission: per-engine FIFOs never stall on bank WARs.
"""

import numpy as np
import ml_dtypes
from contextlib import ExitStack

import concourse.bass as bass
from concourse import bacc
import concourse.mybir as mybir
import concourse.tile as tile
from concourse.bass_utils import run_bass_kernel_spmd

F32 = mybir.dt.float32
F32R = mybir.dt.float32r
BF16 = mybir.dt.bfloat16
I32 = mybir.dt.int32
AF = mybir.ActivationFunctionType
OP = mybir.AluOpType

B, C, H, W = 8, 512, 32, 32
L = H * W  # 1024 query pixels
LC = 1024  # context tokens
HEADS, HD = 8, 64
VW = HD + 1  # 65: v columns + ones column (emits softmax denominator)
HID = HEADS * HD  # 512
EPS = 1e-6
NCORES = 8
CT = C // 128  # 4 c-tiles
JT = LC // 128  # 8 j-tiles

MAGIC = 0x5F3759DF


def build():
    nc = bacc.Bacc("TRN2", target_bir_lowering=False, debug=False,
                   num_devices=NCORES)

    x_d = nc.dram_tensor("x", [C, L], BF16, kind="ExternalInput")
    ct_d = nc.dram_tensor("ctxT", [C, LC], BF16, kind="ExternalInput")
    wq_d = nc.dram_tensor("wq", [C, HID], BF16, kind="ExternalInput")
    wk_d = nc.dram_tensor("wk", [C, HID], BF16, kind="ExternalInput")
    wv_d = nc.dram_tensor("wv", [C, HID], BF16, kind="ExternalInput")
    wo_d = nc.dram_tensor("wo", [HID, C], BF16, kind="ExternalInput")
    ones_d = nc.dram_tensor("ones", [128, 128], F32R, kind="ExternalInput")
    ident_d = nc.dram_tensor("ident", [128, 128], F32R, kind="ExternalInput")
    bog2_d = nc.dram_tensor("bog2T", [2, C], F32R, kind="ExternalInput")
    y_d = nc.dram_tensor("y_out", [C, L], F32, kind="ExternalOutput")

    with tile.TileContext(nc) as tc, ExitStack() as top:
        pc = top.enter_context(tc.tile_pool(name="main", bufs=1))
        psum = top.enter_context(tc.tile_pool(name="ps", bufs=1, space="PSUM"))

        # ---------------- input DMAs (spread across rings) ----------------
        ct_sb, x_sb = [], []
        for t in range(CT):
            ctt = pc.tile([128, LC], BF16, tag=f"ct{t}")
            nc.sync.dma_start(out=ctt, in_=ct_d[t * 128:(t + 1) * 128, :])
            ct_sb.append(ctt)
        for t in range(CT):
            xt = pc.tile([128, L], BF16, tag=f"x{t}")
            nc.scalar.dma_start(out=xt, in_=x_d[t * 128:(t + 1) * 128, :])
            x_sb.append(xt)
        wk_sb, wq_sb, wv_sb = [], [], []
        for name, lst, dram in (("wk", wk_sb, wk_d), ("wq", wq_sb, wq_d),
                                ("wv", wv_sb, wv_d)):
            for t in range(CT):
                wt = pc.tile([128, HID], BF16, tag=f"{name}{t}")
                nc.scalar.dma_start(out=wt, in_=dram[t * 128:(t + 1) * 128, :])
                lst.append(wt)
        ones_sb = pc.tile([128, 128], F32R, tag="ones")
        nc.gpsimd.dma_start(out=ones_sb, in_=ones_d[:, :])
        ident_sb = pc.tile([128, 128], F32R, tag="ident")
        nc.gpsimd.dma_start(out=ident_sb, in_=ident_d[:, :])
        bog2_sb = pc.tile([2, C], F32R, tag="bog2")
        nc.gpsimd.dma_start(out=bog2_sb, in_=bog2_d[:, :])
        wo_sb = []
        for t in range(CT):
            wt = pc.tile([128, C], BF16, tag=f"wo{t}")
            nc.gpsimd.dma_start(out=wt, in_=wo_d[t * 128:(t + 1) * 128, :])
            wo_sb.append(wt)

        # ---------------- PE warmup (runs during DMA wait) ----------------
        warm_sb = pc.tile([128, 512], F32, tag="warm")
        nc.vector.memset(warm_sb, 1.0)
        warm_ps = psum.tile([128, 512], F32, tag="spare", name="warmps",
                            bufs=2)
        for i in range(10):
            nc.tensor.matmul(out=warm_ps[:, :],
                             lhsT=warm_sb[:, 0:128].bitcast(F32R),
                             rhs=warm_sb[:, :].bitcast(F32R),
                             start=True, stop=True)
        warm_ex = pc.tile([1, 8], F32R, tag="warmex")
        nc.scalar.activation(out=warm_ex[:, :], in_=warm_sb[0:1, 0:8],
                             func=AF.Exp, bias=0.0, scale=0.0)

        # ---------------- squares + transposed stats -----------------------
        # ssq cols (pairs, col 2c used): 0:16 x-pixel chunks, 16:32 ctx
        ssq_ps = psum.tile([128, 512], F32, tag="sim", name="ssqps", bufs=2)
        sq_x, sq_c = [], []
        for t in range(CT):
            s = pc.tile([128, L], F32R, tag="sq", name=f"sqx{t}", bufs=4)
            nc.vector.tensor_mul(s[:, :], x_sb[t][:, :], x_sb[t][:, :])
            sq_x.append(s)
        for t in range(CT):
            s = pc.tile([128, LC], F32R, tag="sq", name=f"sqc{t}", bufs=4)
            nc.vector.tensor_mul(s[:, :], ct_sb[t][:, :], ct_sb[t][:, :])
            sq_c.append(s)
        for c in range(8):
            for t in range(CT):
                nc.tensor.matmul(out=ssq_ps[:, 2 * c:2 * c + 2],
                                 lhsT=sq_x[t][:, c * 128:(c + 1) * 128],
                                 rhs=ones_sb[:, 0:2],
                                 start=(t == 0), stop=(t == CT - 1))
        for c in range(8):
            for t in range(CT):
                nc.tensor.matmul(out=ssq_ps[:, 16 + 2 * c:18 + 2 * c],
                                 lhsT=sq_c[t][:, c * 128:(c + 1) * 128],
                                 rhs=ones_sb[:, 0:2],
                                 start=(t == 0), stop=(t == CT - 1))

        # Quake rsqrt on DVE: dst = (src/nfeat + eps)^-0.5
        kmagic = pc.tile([128, 32], I32, tag="kmagic")
        nc.vector.memset(kmagic, MAGIC)

        def dve_rsqrt(dst, src_ps, ncols, nfeat, scratch_tag):
            m = pc.tile([128, ncols], F32, tag=f"{scratch_tag}m")
            nc.vector.tensor_scalar(out=m[:, :], in0=src_ps[:, 0:ncols],
                                    scalar1=1.0 / nfeat, scalar2=EPS,
                                    op0=OP.mult, op1=OP.add)
            m2 = pc.tile([128, ncols], F32, tag=f"{scratch_tag}m2")
            nc.vector.tensor_scalar(out=m2[:, :], in0=src_ps[:, 0:ncols],
                                    scalar1=0.5 / nfeat, scalar2=0.5 * EPS,
                                    op0=OP.mult, op1=OP.add)
            sh = pc.tile([128, ncols], I32, tag=f"{scratch_tag}sh")
            nc.vector.tensor_scalar(out=sh[:, :],
                                    in0=m[:, :].bitcast(I32),
                                    scalar1=1, scalar2=0,
                                    op0=OP.logical_shift_right,
                                    op1=OP.logical_shift_right)
            y0 = pc.tile([128, ncols], F32, tag=f"{scratch_tag}y0")
            nc.vector.scalar_tensor_tensor(
                out=y0[:, :].bitcast(I32), in0=kmagic[:, 0:ncols], scalar=0,
                in1=sh[:, :], op0=OP.add, op1=OP.subtract)
            # 2 Newton iters, negated form (signs cancel):
            # y' = (m2*y^2 - 1.5) * y
            t1 = pc.tile([128, ncols], F32, tag=f"{scratch_tag}t1")
            y1 = pc.tile([128, ncols], F32, tag=f"{scratch_tag}y1")
            nc.vector.tensor_mul(t1[:, :], y0[:, :], y0[:, :])
            nc.vector.tensor_mul(t1[:, :], t1[:, :], m2[:, :])
            nc.vector.scalar_tensor_tensor(
                out=y1[:, :], in0=t1[:, :], scalar=1.5, in1=y0[:, :],
                op0=OP.subtract, op1=OP.mult)
            nc.vector.tensor_mul(t1[:, :], y1[:, :], y1[:, :])
            nc.vector.tensor_mul(t1[:, :], t1[:, :], m2[:, :])
            nc.vector.scalar_tensor_tensor(
                out=dst[:, :], in0=t1[:, :], scalar=1.5, in1=y1[:, :],
                op0=OP.subtract, op1=OP.mult)

        # ---------------- projection machinery -----------------------------
        q_sb = [pc.tile([128, L], F32R, tag=f"q{m}", name=f"q{m}")
                for m in range(CT)]
        k_sb = [pc.tile([128, LC], F32R, tag=f"k{m}", name=f"k{m}")
                for m in range(CT)]
        vT_sb = []
        for j in range(JT):
            vt = pc.tile([128, HEADS * VW], F32R, tag=f"vT{j}", name=f"vT{j}")
            vh = vt[:, :].rearrange("p (h c) -> p h c", h=HEADS)
            nc.vector.memset(vh[:, :, HD:VW].bitcast(F32), 1.0)
            vT_sb.append(vt)
        ao_sb = [pc.tile([128, L], BF16, tag=f"ao{m}", name=f"ao{m}")
                 for m in range(CT)]
        rsq_xc = pc.tile([128, 32], F32, tag="rsqxc")
        bcx_sb = pc.tile([128, L], F32R, tag="bcx")

        def proj_q(m, n, ptag):
            ns = slice(n * 512, (n + 1) * 512)
            ps = psum.tile([128, 512], F32, tag=ptag, name=f"qp{m}{n}",
                           bufs=2)
            for t in range(CT):
                nc.tensor.matmul(out=ps[:, :],
                                 lhsT=wq_sb[t][:, m * 128:(m + 1) * 128],
                                 rhs=x_sb[t][:, ns],
                                 start=(t == 0), stop=(t == CT - 1))
            nc.vector.tensor_mul(q_sb[m][:, ns], ps[:, :],
                                 bcx_sb[:, ns].bitcast(F32))

        def proj_k(m, h, ptag):
            hs = slice(h * 512, (h + 1) * 512)
            ps = psum.tile([128, 512], F32, tag=ptag, name=f"kp{m}{h}",
                           bufs=2)
            for t in range(CT):
                nc.tensor.matmul(out=ps[:, :],
                                 lhsT=wk_sb[t][:, m * 128:(m + 1) * 128],
                                 rhs=ct_sb[t][:, hs],
                                 start=(t == 0), stop=(t == CT - 1))
            nc.vector.tensor_copy(k_sb[m][:, hs], ps[:, :])

        def proj_v(j, ptag):
            ps = psum.tile([128, HID], F32, tag=ptag, name=f"vp{j}",
                           bufs=2)
            for t in range(CT):
                nc.tensor.matmul(out=ps[:, :],
                                 lhsT=ct_sb[t][:, j * 128:(j + 1) * 128],
                                 rhs=wv_sb[t][:, :],
                                 start=(t == 0), stop=(t == CT - 1))
            vh = vT_sb[j][:, :].rearrange("p (h c) -> p h c", h=HEADS)
            # fold per-token rms factor rc into v
            nc.vector.tensor_scalar_mul(
                vh[:, :, 0:HD],
                ps[:, :].rearrange("p (h c) -> p h c", h=HEADS),
                rsq_xc[:, 16 + 2 * j:17 + 2 * j])

        # k projections first on the PE queue (only need ctx + wk DMAs)
        proj_k(0, 0, "spare")
        proj_k(0, 1, "ou")
        proj_k(1, 0, "spare")
        proj_k(1, 1, "ou")

        # rsq_xc cols (2c): 0:16 pixels rxT, 16:32 tokens rcT
        dve_rsqrt(rsq_xc, ssq_ps, 32, C, "rs")

        # bc_rx [128, L]: bc_rx[p, i] = rx[i] via diag trick
        diag_t = [pc.tile([128, 128], F32R, tag="diag", name=f"dg{c}", bufs=2)
                  for c in range(8)]
        bcx_ps = psum.tile([128, L], F32, tag="sim", name="bcxps", bufs=2)
        for c in range(8):
            nc.vector.tensor_scalar_mul(diag_t[c][:, :],
                                        ident_sb[:, :].bitcast(F32),
                                        rsq_xc[:, 2 * c:2 * c + 1])
            nc.tensor.matmul(out=bcx_ps[:, c * 128:(c + 1) * 128],
                             lhsT=ones_sb[:, :], rhs=diag_t[c][:, :],
                             start=True, stop=True)
        nc.vector.tensor_copy(bcx_sb[:, :], bcx_ps[:, :])

        # bog2 "transpose": [2, C] row layout -> [128, 2] per c-tile
        bo_sb, g2_sb = [], []
        for t in range(CT):
            bps = psum.tile([128, 512], F32, tag="ou", name=f"bog{t}", bufs=2)
            nc.tensor.matmul(out=bps[:, 0:2],
                             lhsT=bog2_sb[:, t * 128:(t + 1) * 128],
                             rhs=ident_sb[0:2, 0:2],
                             start=True, stop=True)
            bg = pc.tile([128, 2], F32, tag=f"bog2s{t}")
            nc.vector.tensor_copy(bg[:, :], bps[:, 0:2])
            bo_sb.append(bg[:, 0:1])
            g2_sb.append(bg[:, 1:2])

        # rest of the pre-attention projections
        proj_q(0, 0, "spare")
        proj_q(1, 0, "ou")
        proj_v(0, "spare")
        proj_v(1, "ou")
        proj_v(2, "spare")
        proj_v(3, "ou")
        proj_v(4, "spare")
        proj_v(5, "ou")

        # deferred projection work, drained into stage-C PE slack
        filler = []
        filler.append(lambda: proj_v(6, "spare"))
        filler.append(lambda: proj_v(7, "spare"))
        filler.append(lambda: proj_k(2, 0, "spare"))
        filler.append(lambda: proj_k(2, 1, "spare"))
        filler.append(lambda: proj_k(3, 0, "spare"))
        filler.append(lambda: proj_k(3, 1, "spare"))
        filler.append(lambda: proj_q(2, 0, "spare"))
        filler.append(lambda: proj_q(3, 0, "spare"))
        for m in range(CT):
            filler.append(lambda m=m: proj_q(m, 1, "spare"))

        # ---------------- stage D (emitted later, per n) --------------------
        xf32 = [pc.tile([128, L], F32, tag=f"xf{t}", name=f"xf{t}")
                for t in range(CT)]

        def emit_xf32():
            for t in range(CT):
                nc.gpsimd.tensor_copy(xf32[t][:, :], x_sb[t][:, :])
        ybig = pc.tile([128, 4 * L], F32, tag="ybig")
        ysq_t = [pc.tile([128, 512], F32R, tag=f"ysq{m}", name=f"ysq{m}")
                 for m in range(CT)]

        def stage_d(n):
            ns = slice(n * 512, (n + 1) * 512)
            ops = []
            for m in range(CT):
                def dproj(m=m):
                    ps = psum.tile([128, 512], F32, tag="spare",
                                   name=f"yp{m}{n}", bufs=2)
                    for t in range(CT):
                        nc.tensor.matmul(
                            out=ps[:, :],
                            lhsT=wo_sb[t][:, m * 128:(m + 1) * 128],
                            rhs=ao_sb[t][:, ns],
                            start=(t == 0), stop=(t == CT - 1))
                    ysl = ybig[:, m * L + n * 512: m * L + (n + 1) * 512]
                    nc.vector.tensor_scalar_add(ysl, ps[:, :], bo_sb[m])
                    nc.gpsimd.tensor_mul(ysq_t[m][:, :], ysl, ysl)
                ops.append(dproj)

            def dstat():
                ssy = psum.tile([128, 512], F32, tag="sim", bufs=2,
                                name=f"ssy{n}")
                for c in range(4):
                    for m in range(CT):
                        nc.tensor.matmul(
                            out=ssy[:, 2 * c:2 * c + 2],
                            lhsT=ysq_t[m][:, c * 128:(c + 1) * 128],
                            rhs=ones_sb[:, 0:2],
                            start=(m == 0), stop=(m == CT - 1))
                ry = pc.tile([128, 8], F32, tag=f"ry{n}")
                dve_rsqrt(ry, ssy, 8, C, f"ry{n}")
                bcy = psum.tile([128, 512], F32, tag="spare", name=f"bcy{n}",
                                bufs=2)
                for c in range(4):
                    dg = pc.tile([128, 128], F32R, tag="diag",
                                 name=f"dgy{n}{c}", bufs=2)
                    nc.vector.tensor_scalar_mul(dg[:, :],
                                                ident_sb[:, :].bitcast(F32),
                                                ry[:, 2 * c:2 * c + 1])
                    nc.tensor.matmul(out=bcy[:, c * 128:(c + 1) * 128],
                                     lhsT=ones_sb[:, :], rhs=dg[:, :],
                                     start=True, stop=True)
                for m in range(CT):
                    ysl = ybig[:, m * L + n * 512: m * L + (n + 1) * 512]
                    tmp = pc.tile([128, 512], F32, tag="fintmp",
                                  name=f"ft{n}{m}", bufs=2)
                    nc.vector.scalar_tensor_tensor(
                        out=tmp[:, :], in0=ysl, scalar=g2_sb[m],
                        in1=bcy[:, :], op0=OP.mult, op1=OP.mult)
                    nc.gpsimd.tensor_add(ysl, tmp[:, :], xf32[m][:, ns])
                    nc.sync.dma_start(
                        out=y_d[m * 128:(m + 1) * 128, ns], in_=ysl)
            ops.append(dstat)
            return ops

        # ---------------- stage C: attention -------------------------------
        pexp = top.enter_context(tc.tile_pool(name="exp", bufs=1))

        steps = [(n, p, j) for n in range(2) for p in range(4)
                 for j in range(JT)]

        sim_slots = {}
        ex_slots = {}

        def emit_sims(step):
            n, p, j = step
            ns = slice(n * 512, (n + 1) * 512)
            js = slice(j * 128, (j + 1) * 128)
            sl = psum.tile([128, 1024], F32, tag="sim", bufs=2,
                           name=f"sim{n}{p}{j}")
            nc.tensor.matmul(out=sl[:, 0:512],
                             lhsT=k_sb[p][0:64, js],
                             rhs=q_sb[p][0:64, ns],
                             start=True, stop=True)
            nc.tensor.matmul(out=sl[:, 512:1024],
                             lhsT=k_sb[p][64:128, js],
                             rhs=q_sb[p][64:128, ns],
                             start=True, stop=True)
            sim_slots[step] = sl

        def emit_exps(step):
            n, p, j = step
            ex = pexp.tile([128, 1024], F32R, tag="ex", bufs=4,
                           name=f"ex{n}{p}{j}")
            nc.scalar.activation(out=ex[:, :], in_=sim_slots[step][:, :],
                                 func=AF.Exp, bias=0.0,
                                 scale=rsq_xc[:, 16 + 2 * j:17 + 2 * j])
            ex_slots[step] = ex

        ou_cur = {}

        def emit_pv(step):
            n, p, j = step
            if j == 0:
                ou_cur[0] = psum.tile([128, 512], F32, tag="ou", bufs=2,
                                      name=f"ou{n}{p}0")
                ou_cur[1] = psum.tile([128, 512], F32, tag="ou", bufs=2,
                                      name=f"ou{n}{p}1")
            ex = ex_slots[step]
            for hi in range(2):
                h = 2 * p + hi  # global head
                nc.tensor.matmul(
                    out=ou_cur[hi][0:VW, :],
                    lhsT=vT_sb[j][:, h * VW:(h + 1) * VW],
                    rhs=ex[:, hi * 512:(hi + 1) * 512],
                    start=(j == 0), stop=(j == JT - 1))

        def emit_pair_end(step):
            n, p, j = step
            ns = slice(n * 512, (n + 1) * 512)
            for hi in range(2):
                osb = pc.tile([VW, 512], F32R, tag="ousb",
                              name=f"osb{n}{p}{hi}", bufs=4)
                nc.vector.tensor_copy(osb[:, :], ou_cur[hi][0:VW, :])
                bcr = psum.tile([128, 512], F32, tag="spare", bufs=2,
                                name=f"bcr{n}{p}{hi}")
                nc.tensor.matmul(out=bcr[0:HD, :],
                                 lhsT=ones_sb[64:65, 0:HD],
                                 rhs=osb[HD:VW, :],
                                 start=True, stop=True)
                rbc = pc.tile([64, 512], F32, tag="rbc",
                              name=f"rbc{n}{p}{hi}", bufs=2)
                nc.vector.reciprocal_approx_fast(out=rbc[:, :],
                                                 in_=bcr[0:HD, :])
                nc.gpsimd.tensor_mul(
                    ao_sb[p][hi * HD:(hi + 1) * HD, ns],
                    osb[0:HD, :].bitcast(F32), rbc[:, :])

        # ---- emission with software pipelining ----
        d_ops = []
        emit_sims(steps[0])
        for si, step in enumerate(steps):
            n, p, j = step
            if si == 2:
                emit_xf32()
            emit_exps(step)
            if si + 1 < len(steps):
                emit_sims(steps[si + 1])
            emit_pv(step)
            if j == JT - 1:
                emit_pair_end(step)
                if (n, p) == (0, 3):
                    d_ops = stage_d(0)
                elif (n, p) == (1, 3):
                    for op in stage_d(1):
                        op()
            # drain deferred work into PE slack: one PSUM-serial group
            # every other step so the PE FIFO never stalls on a bank WAR
            if si % 2 == 1:
                if filler:
                    filler.pop(0)()
                elif d_ops and si >= 34:
                    d_ops.pop(0)()
        for op in d_ops:
            op()

    nc.compile()
    return nc


_NC_CACHE = {}


def _get_nc():
    if "nc" not in _NC_CACHE:
        _NC_CACHE["nc"] = build()
    return _NC_CACHE["nc"]


def kernel(x, context, Wq, Wkv, Wo, bo, g, g2):
    x = np.asarray(x, dtype=np.float32)
    context = np.asarray(context, dtype=np.float32)
    Wq = np.asarray(Wq, dtype=np.float32)
    Wkv = np.asarray(Wkv, dtype=np.float32)
    Wo = np.asarray(Wo, dtype=np.float32)
    bo = np.asarray(bo, dtype=np.float32)
    g = np.asarray(g, dtype=np.float32)
    g2 = np.asarray(g2, dtype=np.float32)

    bf = ml_dtypes.bfloat16
    scale = HD ** -0.5
    wq_h = np.ascontiguousarray((Wq * g[None, :] * scale).T).astype(bf)
    wk_h = np.ascontiguousarray((Wkv[:HID] * g[None, :]).T).astype(bf)
    wv_h = np.ascontiguousarray((Wkv[HID:] * g[None, :]).T).astype(bf)
    wo_h = np.ascontiguousarray(Wo.T).astype(bf)
    bog2T = np.ascontiguousarray(np.stack([bo, g2], axis=0))  # [2, C]
    ones = np.ones((128, 128), dtype=np.float32)
    ident = np.eye(128, dtype=np.float32)

    nc = _get_nc()
    global _last_in_maps
    in_maps = []
    for i in range(NCORES):
        in_maps.append({
            "x": np.ascontiguousarray(x[i].reshape(C, L)).astype(bf),
            "ctxT": np.ascontiguousarray(context[i].T).astype(bf),
            "wq": wq_h, "wk": wk_h, "wv": wv_h, "wo": wo_h,
            "ones": ones, "ident": ident, "bog2T": bog2T,
        })
    _last_in_maps = in_maps
    res = run_bass_kernel_spmd(nc, in_maps, list(range(NCORES)))
    out = np.stack([res.results[i]["y_out"].reshape(C, H, W)
                    for i in range(NCORES)])
    return out.astype(np.float32)


_last_in_maps = None
